# revision 54
# baseline (speedup 1.0000x reference)
"""Trainium2 Bass kernel for 3D conv-attention layer (v2, bf16 + algebraic fusion).

Reference (per (b,h,w) "site", D=32 positions, S=32 features):
  k,q,v = 1x1 conv of x [B,C,D,H,W] -> [B,S,D,H,W]
  scoresT[j,i] = sum_s q[s,j] k[s,i] / sqrt(S)   (per site)
  aT = softmax over i  (free dim of scoresT)
  o[s,j] = sum_i v[s,i] a[i,j];   y = x + Wo @ o + bo

Key algebra (removes k/q projections AND the operand-colocation problem):
  scoresT/sqrt(S) = X~^T @ (G~ @ X)  per site, where
    G2 = Wq^T Wk / sqrt(S)  [C,C],  g = Wk^T bq / sqrt(S)  [C]
    G~ = [[G2],[g^T]] [C+1,C],  X~ = [X; ones] [C+1,D]
  (all j-only / const score terms cancel in the softmax over i;
   bv folds into a constant output bias since sum_i a[i,j] == 1:
   b* = Wo bv + bo, pre-added to x on the host.)

Cost-model-aware choices (TimelineSim charges matmuls by OUTPUT FREE SIZE
only, at 1 cycle/row for bf16; DVE/ACT ops by max free size):
  - everything bf16 into the PE, fp32 PSUM accumulation
  - P2 = G~X as 4 [65,512] matmuls/chunk (2048 rows)
  - Wo folded into v: uT = X^T (Wo Wv)^T computed BY THE PE (32 pair-
    matmuls, 2048 rows) -> z = uT^T a goes STRAIGHT into the y PSUM,
    eliminating the separate o matmuls, o drain, and out-projection
  - scores/z: 64 [32,32]/[64,32] matmuls each, stacked 4 sites per
    128-partition PSUM tile so softmax runs as [128,512]-wide ops
  - single DVE block-transpose (aT -> a) per chunk; no StreamTranspose
    for v/u (it has no fast DVE modes)
  - host pre-transposes x so every DMA is >=512B-contiguous per partition

Sharding: data-parallel over H (8 cores x 8 rows).
Per core: 32 chunks of (b,h), each 64 sites of [C=64, D=32].
"""

import math
from contextlib import ExitStack

import numpy as np
import ml_dtypes

import concourse.bass as bass
import concourse.mybir as mybir
from concourse import bacc
import concourse.tile as tile
from concourse.bass_utils import run_bass_kernel_spmd

B, C, D, H, W = 4, 64, 32, 64, 64
S = C // 2  # 32
NCORES = 8
HS = H // NCORES  # 8
F32 = mybir.dt.float32
BF16 = mybir.dt.bfloat16
FD = D * W  # 2048 free elems per chunk


def mkap(base, part0, pcount, foff, fdims):
    """AP at partition block [part0, part0+pcount) of a tile, free offset foff,
    free dims [(step, count), ...] in the tile's flat free space."""
    full = base[...] if not isinstance(base, bass.AP) else base
    pstride = full.ap[0][0]
    return bass.AP(tensor=full.tensor,
                   offset=full.offset + part0 * pstride + foff,
                   ap=[[pstride, pcount]] + [list(d) for d in fdims])


YDT = BF16  # output dtype (bf16 halves the store DMA)



def _copy(nc, eng, out, in_):
    if eng == "act":
        nc.scalar.activation(out, in_, mybir.ActivationFunctionType.Copy)
    elif eng == "dve":
        nc.vector.tensor_copy(out=out, in_=in_)
    else:
        nc.gpsimd.tensor_copy(out=out, in_=in_)


def _add(nc, eng, out, in0, in1):
    e = nc.vector if eng == "dve" else nc.gpsimd
    e.tensor_tensor(out=out, in0=in0, in1=in1, op=mybir.AluOpType.add)

def build_program(ydt=YDT, xt_bufs=4, xb_bufs=3, pdr_bufs=2, sb_bufs=32,
                  A_bufs=1, ut_bufs=None, warmup=0,
                  y_bufs=3, pj_bufs=3, vt_bufs=1, ab_bufs=1, yps_bufs=1,
                  probe=(), pd_eng=("act", "act", "act", "act"),
                  vtsb_eng=("act", "dve"), odr_eng=("act", "dve"),
                  fin_eng=("dve", "dve"), norm_eng="split",
                  sc_order="tmaj", layout="single", order="loso",
                  fin_prio=0, nsplit=11):
    if ut_bufs is None:
        ut_bufs = sb_bufs
    nc = bacc.Bacc()
    xt_d = nc.declare_dram_parameter("xt", [B, HS, C + 1, FD], BF16,
                                     isOutput=False)
    xb_d = nc.declare_dram_parameter("xb", [B, HS, 128, 1024], BF16,
                                     isOutput=False)
    L_d = nc.declare_dram_parameter("L", [C, C + 1], BF16, isOutput=False)
    wu_d = nc.declare_dram_parameter("wuT", [C, C], BF16, isOutput=False)
    y_d = nc.declare_dram_parameter("y", [B, HS, 128, 1024], ydt,
                                    isOutput=True)

    EXP = mybir.ActivationFunctionType.Exp
    CPY = mybir.ActivationFunctionType.Copy

    with tile.TileContext(nc) as tc, ExitStack() as ctx:
        const = ctx.enter_context(tc.tile_pool(name="const", bufs=1))
        xtp = ctx.enter_context(tc.tile_pool(name="xtp", bufs=xt_bufs))
        xbp = ctx.enter_context(tc.tile_pool(name="xbp", bufs=xb_bufs))
        pdrp = ctx.enter_context(tc.tile_pool(name="pdrp", bufs=pdr_bufs))
        sbp = ctx.enter_context(tc.tile_pool(name="sbp", bufs=sb_bufs))
        ysp = ctx.enter_context(tc.tile_pool(name="ysp", bufs=y_bufs))
        pj_ps = ctx.enter_context(tc.tile_pool(name="pj_ps", bufs=pj_bufs,
                                               space="PSUM"))
        ab_ps = ctx.enter_context(tc.tile_pool(name="ab_ps", bufs=ab_bufs,
                                               space="PSUM"))
        o_ps = ctx.enter_context(tc.tile_pool(name="o_ps", bufs=vt_bufs,
                                              space="PSUM"))
        y_ps = ctx.enter_context(tc.tile_pool(name="y_ps", bufs=yps_bufs,
                                              space="PSUM"))

        # ---- constants ----
        L_sb = const.tile([C, C + 1], BF16, tag="L")
        nc.sync.dma_start(out=L_sb[:, :], in_=L_d[:, :])
        wu_sb = const.tile([C, C], BF16, tag="wu")
        nc.sync.dma_start(out=wu_sb[:, :], in_=wu_d[:, :])

        # ---- PE p-state warm-up: ~3us of dummy matmuls on the const
        # tiles, hidden under the first x DMAs, so real matmuls start at
        # the full 2.4GHz clock instead of ramping through it.  They
        # write the first chunk's scores bank, which every real scores
        # matmul resets (start=True) before use.
        if warmup:
            warm_ps = ab_ps.tile([128, 512], F32, tag="A", name="warm",
                                 bufs=A_bufs)
            for wi in range(warmup):
                nc.tensor.matmul(
                    warm_ps[0:64, 0:64], wu_sb[:, :], wu_sb[:, :],
                    start=True, stop=True, tile_position=(0, 0))

        # ---------- software-pipelined chunk emission ----------
        # Engines run their queues in order, so the PE stream must never
        # queue o(i) (which waits on chunk i's softmax chain) ahead of
        # independent work.  Per iteration i the PE sees:
        #   proj/vT(i) | outproj(i-2) | scores(i) | o(i-1)
        # which keeps it busy while softmax(i-1) runs on ACT/DVE.
        chunks = [(b, h) for b in range(B) for h in range(HS)]
        n = len(chunks)
        st = {}  # chunk index -> dict of live tiles

        def stage_load_proj(i):
            b, h = chunks[i]
            s = st[i] = {}
            xt = s["xt"] = xtp.tile([C + 1, FD], BF16, tag="xt", name="xt")
            nc.sync.dma_start(out=xt[:, :], in_=xt_d[b, h, :, :])
            xb = s["xb"] = xbp.tile([128, 1024], BF16, tag="xb", name="xb")
            nc.sync.dma_start(out=xb[:, :], in_=xb_d[b, h, :, :])

            # A/B PSUM banks: cols 0-255 scoresT, 256-511 vT.  Same bank
            # from different tile columns is fine; the parity split keeps
            # same-column matmuls in alternating banks.
            tS = ab_ps.tile([128, 512], F32, tag="A", name="tS",
                            bufs=A_bufs)
            s["tAB"] = tAB = (tS, tS)
            tU = s["tU"] = (ab_ps.tile([128, 512], F32, tag="V", name="tU0"),
                            ab_ps.tile([128, 512], F32, tag="V2",
                                       name="tU1"))

            def ut_mm(p):
                # uT for site pair (2p, 2p+1): [64(2-site i), 64(c)] block
                # at partition half p%2, free slot (p%16)//2, tile p//16;
                # tile_position col 64*(p%2) (proven pattern).
                nc.tensor.matmul(
                    tU[p // 16][64 * (p % 2):64 * (p % 2) + 64,
                                64 * ((p % 16) // 2):
                                64 * ((p % 16) // 2) + 64],
                    xt[0:C, 64 * p:64 * p + 64],
                    wu_sb[:, :], start=True, stop=True,
                    tile_position=(0, 64 * (p % 2)))
            s["ut_mm"] = ut_mm

            # P2 = G~ @ X [65, 512] x4, interleaved with vT = X^T Wv^T
            # matmuls to space same-column same-bank pairs apart.
            pdr = s["pdr"] = []
            for qd in range(4):
                pp = pj_ps.tile([C + 1, 512], F32, tag="pp", name="pp")
                nc.tensor.matmul(pp[:, :], L_sb[:, :],
                                 xt[0:C, 512 * qd:512 * qd + 512],
                                 start=True, stop=True, tile_position=(0, 0))
                ut_mm(2 * qd)
                ut_mm(2 * qd + 1)
                pd = pdrp.tile([C + 1, 512], BF16, tag=f"pd{qd}", name="pd")
                _copy(nc, pd_eng[qd], pd[:, :], pp[:, :])
                pdr.append(pd)

        def stage_scores_softmax(i):
            s = st[i]
            xt, pdr, tAB = s["xt"], s["pdr"], s["tAB"]
            # scoresT: site w=4t+bc -> parity bank by t%2, partitions 32*bc,
            # free 32*(t//2); remaining vT matmuls spaced among the scores.
            for ti, t in enumerate(range(16)):
                sc = tAB[t % 2]
                fo = 32 * t
                # remaining 24 uT pair-matmuls spaced among the scores
                for p in range(8 + (24 * ti) // 16,
                               8 + (24 * (ti + 1)) // 16):
                    s["ut_mm"](p)
                for bc in range(4):
                    w = 4 * t + bc
                    nc.tensor.matmul(
                        sc[32 * bc:32 * bc + 32, fo:fo + 32],
                        xt[0:C + 1, 32 * w:32 * w + 32],
                        pdr[w // 16][0:C + 1,
                                     32 * (w % 16):32 * (w % 16) + 32],
                        start=True, stop=True,
                        tile_position=(0, 32 * bc))

            utdr = s["utdr"] = (
                sbp.tile([128, 512], BF16, tag="ut0", name="ut0",
                         bufs=ut_bufs),
                sbp.tile([128, 512], BF16, tag="ut1", name="ut1",
                         bufs=ut_bufs))
            _copy(nc, vtsb_eng[0], utdr[0][:, :], s["tU"][0][:, :])
            _copy(nc, vtsb_eng[1], utdr[1][:, :], s["tU"][1][:, :])
            e_sb = sbp.tile([128, 512], BF16, tag="e", name="e_sb")
            nc.scalar.activation(e_sb[:, :], tAB[0][:, :], EXP)
            den = sbp.tile([128, 16], F32, tag="den", name="den")
            nc.vector.reduce_sum(
                out=den[:, :],
                in_=mkap(e_sb, 0, 128, 0, [[32, 16], [1, 32]]),
                axis=mybir.AxisListType.X)
            atn = sbp.tile([128, 512], BF16, tag="atn", name="atn")
            if norm_eng == "div":
                nc.vector.tensor_tensor(
                    out=atn[:, :], in0=e_sb[:, :],
                    in1=mkap(den, 0, 128, 0, [[1, 16], [0, 32]]),
                    op=mybir.AluOpType.divide)
            elif norm_eng == "pooldiv":
                nc.gpsimd.tensor_tensor(
                    out=atn[:, :], in0=e_sb[:, :],
                    in1=mkap(den, 0, 128, 0, [[1, 16], [0, 32]]),
                    op=mybir.AluOpType.divide)
            elif norm_eng == "split":
                # halves run concurrently: DVE keeps t<8, the otherwise-idle
                # GPSIMD takes t>=8 (SBUF-only op, so it is Pool-legal);
                # chain latency grows only by the slower half.
                rcp = sbp.tile([128, 16], F32, tag="rcp", name="rcp")
                nc.vector.reciprocal(rcp[:, :], den[:, :])
                nk = 32 * nsplit
                nc.vector.tensor_tensor(
                    out=atn[:, 0:nk], in0=e_sb[:, 0:nk],
                    in1=mkap(rcp, 0, 128, 0, [[1, nsplit], [0, 32]]),
                    op=mybir.AluOpType.mult)
                nc.gpsimd.tensor_tensor(
                    out=atn[:, nk:512], in0=e_sb[:, nk:512],
                    in1=mkap(rcp, 0, 128, nsplit,
                             [[1, 16 - nsplit], [0, 32]]),
                    op=mybir.AluOpType.mult)
            else:
                rcp = sbp.tile([128, 16], F32, tag="rcp", name="rcp")
                nc.vector.reciprocal(rcp[:, :], den[:, :])
                neng = nc.gpsimd if norm_eng == "pool" else nc.vector
                neng.tensor_tensor(
                    out=atn[:, :], in0=e_sb[:, :],
                    in1=mkap(rcp, 0, 128, 0, [[1, 16], [0, 32]]),
                    op=mybir.AluOpType.mult)
            a4 = s["a4"] = sbp.tile([128, 512], BF16, tag="a4", name="a4")
            nc.vector.transpose(a4[:, :], atn[:, :])

        def stage_o(i):
            # z = (Wo Wv) X a straight into the y PSUM pair:
            # site w -> yp[(w%4)//2][64*(w%2)+c, 32*(w//4)+j]
            s = st[i]
            utdr, a4 = s["utdr"], s["a4"]
            yp = s["yp"] = (y_ps.tile([128, 512], F32, tag="y0", name="yp0"),
                            y_ps.tile([128, 512], F32, tag="y1", name="yp1"))
            for t in range(16):
                for bc in range(4):
                    w = 4 * t + bc
                    pb = 32 * (w % 4)
                    nc.tensor.matmul(
                        yp[(w % 4) // 2][64 * (w % 2):64 * (w % 2) + 64,
                                         32 * t:32 * t + 32],
                        utdr[t // 8][pb:pb + 32,
                                     64 * (t % 8):64 * (t % 8) + 64],
                        a4[pb:pb + 32, 32 * t:32 * t + 32],
                        start=True, stop=True,
                        tile_position=(pb, 64 * (w % 2)))

        def stage_out(i):
            b, h = chunks[i]
            s = st[i]
            xb = s["xb"]
            yp0, yp1 = s["yp"]
            y_sb = ysp.tile([128, 1024], ydt, tag="y", name="y_sb")
            if fin_prio:
                with tc.high_priority(offset=fin_prio):
                    _add(nc, fin_eng[0], y_sb[:, 0:512], yp0[:, :],
                         xb[:, 0:512])
                    _add(nc, fin_eng[1], y_sb[:, 512:1024], yp1[:, :],
                         xb[:, 512:1024])
            else:
                _add(nc, fin_eng[0], y_sb[:, 0:512], yp0[:, :], xb[:, 0:512])
                _add(nc, fin_eng[1], y_sb[:, 512:1024], yp1[:, :],
                     xb[:, 512:1024])
            nc.sync.dma_start(out=y_d[b, h, :, :], in_=y_sb[:, :])
            del st[i]

        for i in range(n + 2):
            if order == "loso":
                if i < n:
                    stage_load_proj(i)
                if i >= 2:
                    stage_out(i - 2)
                if i < n:
                    stage_scores_softmax(i)
                if i >= 1 and i - 1 < n:
                    stage_o(i - 1)
            elif order == "lsoo":
                if i < n:
                    stage_load_proj(i)
                if i < n:
                    stage_scores_softmax(i)
                if i >= 1 and i - 1 < n:
                    stage_o(i - 1)
                if i >= 2:
                    stage_out(i - 2)
            else:  # "olso"
                if i >= 1 and i - 1 < n:
                    stage_o(i - 1)
                if i < n:
                    stage_load_proj(i)
                if i >= 2:
                    stage_out(i - 2)
                if i < n:
                    stage_scores_softmax(i)

    nc.finalize()
    return nc


_NC_CACHE = {}


def get_nc():
    if "nc" not in _NC_CACHE:
        _NC_CACHE["nc"] = build_program()
    return _NC_CACHE["nc"]


def make_in_maps(x, Wk, bk, Wq, bq, Wv, bv, Wo, bo):
    f = np.float32
    bf = ml_dtypes.bfloat16
    x = np.asarray(x, f)
    Wk, bk = np.asarray(Wk, f), np.asarray(bk, f)
    Wq, bq = np.asarray(Wq, f), np.asarray(bq, f)
    Wv, bv = np.asarray(Wv, f), np.asarray(bv, f)
    Wo, bo = np.asarray(Wo, f), np.asarray(bo, f)

    isq = 1.0 / math.sqrt(S)
    G2 = (Wq.T @ Wk) * isq                     # [C, C]
    g = (Wk.T @ bq) * isq                      # [C]
    Gt = np.concatenate([G2, g[None, :]], 0)   # [C+1, C]
    L = np.ascontiguousarray(Gt.T).astype(bf)  # [C, C+1] lhsT
    wuT = np.ascontiguousarray((Wo @ Wv).T).astype(bf)   # [C, C]
    bst = Wo @ bv + bo                         # [C]

    # xt: [B, H, C+1, W*D] bf16, free index = 32*w + d, ones row at c=C
    xt_full = np.empty((B, H, C + 1, FD), dtype=bf)
    xw = np.transpose(x, (0, 3, 1, 4, 2))      # [B, H, C, W, D]
    xt_full[:, :, :C, :] = xw.reshape(B, H, C, FD).astype(bf)
    xt_full[:, :, C, :] = np.float32(1.0)

    # xb: [B, H, 128, 1024] bf16:
    #   partition 64*sg + c, free 512*hf + 32*t + d, site w = 4*t + 2*hf + sg
    xbv = x + bst[None, :, None, None, None]
    # [B, H, C, W, D] -> split w = 4*t + 2*hf + sg -> [B, H, sg, c, hf, t, d]
    xb6 = np.transpose(xbv, (0, 3, 1, 4, 2)).reshape(B, H, C, W // 4, 2, 2, D)
    # axes: b, h, c, t, hf, sg, d -> want [b, h, sg, c, hf, t, d]
    xb_full = np.ascontiguousarray(
        np.transpose(xb6, (0, 1, 5, 2, 4, 3, 6))).reshape(
            B, H, 128, 1024).astype(bf)

    in_maps = []
    for i in range(NCORES):
        sl = slice(i * HS, (i + 1) * HS)
        m = {
            "xt": np.ascontiguousarray(xt_full[:, sl]),
            "xb": np.ascontiguousarray(xb_full[:, sl]),
            "L": L, "wuT": wuT,
        }
        in_maps.append(m)
    return in_maps


def gather(results):
    out = np.empty((B, C, D, H, W), dtype=np.float32)
    for i in range(NCORES):
        yr = np.asarray(results[i]["y"], dtype=np.float32)  # [B,HS,128,1024]
        y7 = yr.reshape(B, HS, 2, 64, 2, 16, 32)  # b,h,sg,c,hf,t,d
        # -> [B, C, D, HS, t, hf, sg] then w = 4t + 2hf + sg
        yw = np.transpose(y7, (0, 3, 6, 1, 5, 4, 2)).reshape(
            B, 64, 32, HS, W)
        out[:, :, :, i * HS:(i + 1) * HS, :] = yw
    return out


def kernel(x, Wk, bk, Wq, bq, Wv, bv, Wo, bo):
    nc = get_nc()
    in_maps = make_in_maps(x, Wk, bk, Wq, bq, Wv, bv, Wo, bo)
    res = run_bass_kernel_spmd(nc, in_maps, core_ids=list(range(NCORES)))
    return gather(res.results)
